# revision 33
# baseline (speedup 1.0000x reference)
"""Multi-head self-attention with RoPE on 8 Trainium2 NeuronCores.

Sharding: data-parallel over batch (2) x tensor-parallel over heads (4 groups
of 4 heads). Each core computes its heads' attention plus a partial output
projection (row-sharded Wo); the host sums the 4 partials per batch.

v2 design (bf16 matmul inputs, f32 PSUM accumulation). The kernel is PE-bound
(~105us of matmul column-time at 0.42ns/col); everything else is scheduled to
keep PE busy end to end and to start the ACT exp stream as early as possible:
  - Phase 1 interleaves, per x d-tile as its first half arrives from HBM:
    V-proj for s-tiles 0-3 + the Q/K e0-block projections for chunks 0,1
    (RoPE per chunk, swap DMAs on the DVE queue). First exp fires ~15us.
  - Each head's attention is split into an A-pass (kb 0-7 x q in [q0,1024):
    complete attention for q-tiles 0-7, needs only chunks 0/1 of Q/K e0|e1)
    and a B-pass (kb 0-15 x q in [max(1024,q0), 2048): q-tiles 8-15).
    Pass order h0A..h3A, h0B..h3B; remaining projection chunks, V s-tiles
    4-15 (st-outer) and the Wo tails ride as fillers in the exp gaps.
  - Scores transposed PT[k,q] per kb; exp on ACT; AV natural out[q,65] with a
    ones-column in V giving the softmax denominator per q in-partition.
    Causal diag masks run on the idle GPSIMD engine.
  - Normalization batched per pass: one reciprocal [128,4] + one broadcast
    tensor-mul per 4 q-tiles; progressive per-tile in h3B to fuse Wo+store.
  - attn[q,e] PE-transposed per q-tile into atnT for the natural-layout Wo;
    bf16 partial stores (host sums 4 partials per batch).
  - PSUM budget: A-passes 3 stripe banks + 2 avq + 2 filler; B wide passes
    4 stripe + 2 avq + 2 filler; h2B/h3B 3 stripe + 2 avq + 3 tail (tp+po).
"""
import os
import sys

import numpy as np

for _p in ("/opt/trn_rl_repo", "/root/.axon_site/_ro/trn_rl_repo"):
    if os.path.isdir(_p) and _p not in sys.path:
        sys.path.insert(0, _p)
        break

import concourse.bacc as bacc
import concourse.tile as tile
from concourse import mybir
from concourse.bass_utils import run_bass_kernel_spmd

B, S, D, H = 2, 2048, 1024, 16
DK = 64
THETA = 10000.0
NCORES = 8
HPC = H // (NCORES // B)  # heads per core = 4
E = HPC * DK              # local dims per core = 256
DT8 = D // 128            # 8 d-tiles
ST = S // 128             # 16 s-tiles
CH = S // 512             # 4 512-chunks
F32 = mybir.dt.float32
BF = mybir.dt.bfloat16

_cache = {}


def _chunks(a, b):
    """Split [a, b) at absolute multiples of 512 (PSUM bank boundaries)."""
    out = []
    c0 = a
    while c0 < b:
        c1 = min(b, (c0 // 512 + 1) * 512)
        out.append((c0, c1))
        c0 = c1
    return out


def _build_nc():
    nc = bacc.Bacc(
        "TRN2",
        target_bir_lowering=False,
        debug=False,
        enable_asserts=False,
        num_devices=NCORES,
    )

    def mm(out, lhsT, rhs, **kw):
        nc.tensor.matmul(out, lhsT, rhs, **kw)

    # ---- I/O (all packed on host into SBUF-ready [128, *] layouts) ----
    x_d = nc.dram_tensor("x_d", [128, DT8 * S], BF, kind="ExternalInput").ap()
    # wq/wk packed e-major: cols [e*1024 + d*128 + j]
    wq_d = nc.dram_tensor("wq_d", [128, 2 * 1024], BF, kind="ExternalInput").ap()
    wk_d = nc.dram_tensor("wk_d", [128, 2 * 1024], BF, kind="ExternalInput").ap()
    wv_d = nc.dram_tensor("wv_d", [128, DT8 * E], BF, kind="ExternalInput").ap()
    wo_d = nc.dram_tensor("wo_d", [128, 2 * D], BF, kind="ExternalInput").ap()
    cos_d = nc.dram_tensor("cos_d", [128, S], BF, kind="ExternalInput").ap()
    sin_d = nc.dram_tensor("sin_d", [128, S], BF, kind="ExternalInput").ap()
    trid_d = nc.dram_tensor("trid_d", [128, 384], BF, kind="ExternalInput").ap()
    out_d = nc.dram_tensor("out", [S, D], BF, kind="ExternalOutput").ap()

    with tile.TileContext(nc) as tc:
      with (
          tc.tile_pool(name="const", bufs=1) as cp,
          tc.tile_pool(name="pers", bufs=1) as pp,
      ):
        cos = cp.tile([128, S], BF, name="cos", tag="cos")
        sin = cp.tile([128, S], BF, name="sin", tag="sin")
        trid = cp.tile([128, 384], BF, name="trid", tag="trid")
        wq_sb = pp.tile([128, 2 * 1024], BF, name="wq", tag="wq")
        wk_sb = pp.tile([128, 2 * 1024], BF, name="wk", tag="wk")
        wv_sb = pp.tile([128, DT8 * E], BF, name="wv", tag="wv")
        wo_sb = pp.tile([128, 2 * D], BF, name="wo", tag="wo")
        x_sb = [pp.tile([128, S], BF, name=f"x{d}", tag=f"x{d}") for d in range(DT8)]
        # per-512-chunk q/k tiles so scores only wait on the chunks they read
        qtc = [[pp.tile([128, 512], BF, name=f"qt{e}c{c}", tag=f"qt{e}c{c}")
                for c in range(CH)] for e in range(2)]
        ktc = [[pp.tile([128, 512], BF, name=f"kt{e}c{c}", tag=f"kt{e}c{c}")
                for c in range(CH)] for e in range(2)]
        v_all = pp.tile([128, ST * (E + HPC)], BF, name="vall", tag="vall")
        attn = pp.tile([128, ST * E], BF, name="attn", tag="attn")

        # ---- input loads, ALL on the SP queue: dma_start holds the issuing
        # queue's SEQ until the shared HWDGE grants it, so putting any input
        # load on the ACT queue would block the exp stream behind the whole
        # input sequence. SP has nothing else to do until the po stores.
        ld = nc.sync.dma_start
        ld(out=wk_sb[:, 0:1024], in_=wk_d[:, 0:1024])
        ld(out=wq_sb[:, 0:1024], in_=wq_d[:, 0:1024])
        ld(out=wv_sb[:, 0:512], in_=wv_d[:, 0:512])
        ld(out=x_sb[0][:, 0:512], in_=x_d[:, 0:512])
        ld(out=wv_sb[:, 512:2048], in_=wv_d[:, 512:2048])
        for d in range(1, DT8):
            ld(out=x_sb[d][:, 0:512], in_=x_d[:, d * S:d * S + 512])
        for d in range(0, 4):
            ld(out=x_sb[d][:, 512:1024], in_=x_d[:, d * S + 512:d * S + 1024])
        ld(out=cos[:, 0:512], in_=cos_d[:, 0:512])
        ld(out=sin[:, 0:512], in_=sin_d[:, 0:512])
        for d in range(4, DT8):
            ld(out=x_sb[d][:, 512:1024], in_=x_d[:, d * S + 512:d * S + 1024])
        ld(out=cos[:, 512:1024], in_=cos_d[:, 512:1024])
        ld(out=sin[:, 512:1024], in_=sin_d[:, 512:1024])
        ld(out=trid, in_=trid_d)
        for d in range(DT8):
            ld(out=x_sb[d][:, 1024:2048], in_=x_d[:, d * S + 1024:(d + 1) * S])
        ld(out=cos[:, 1024:2048], in_=cos_d[:, 1024:2048])
        ld(out=sin[:, 1024:2048], in_=sin_d[:, 1024:2048])
        ld(out=wk_sb[:, 1024:2048], in_=wk_d[:, 1024:2048])
        ld(out=wq_sb[:, 1024:2048], in_=wq_d[:, 1024:2048])
        ld(out=wo_sb, in_=wo_d)

        # ones columns for the softmax denominators
        nc.gpsimd.memset(
            v_all.rearrange("p (st h c) -> p st h c", st=ST, h=HPC)[:, :, :, 64:65],
            1.0)

        vv = v_all.rearrange("p (st h c) -> p st h c", st=ST, h=HPC)
        attn_t = attn.rearrange("p (t e) -> p t e", t=ST)

        with tc.tile_pool(name="rope", bufs=1) as rp:

            def rope_chunk(ps, dstc, c, prpool, prtag):
                """Finish RoPE for one 512-chunk from its raw psum projection:
                dst = ps*cos + rowswap32(ps*sin'). The 32-row-group swap is a
                PE matmul by a constant permutation matrix (trid cols 256:384)
                — no DMA, no HWDGE, no DMA-semaphore latency. tsin/tcos are
                per-call rotating tiles so chunks pipeline independently."""
                sl = slice(c * 512, (c + 1) * 512)
                tsn = rp.tile([128, 512], BF, name="tsn", tag="tsn", bufs=4)
                tcos = rp.tile([128, 512], BF, name="tcos", tag="tcos", bufs=4)
                nc.vector.tensor_mul(out=tsn, in0=ps, in1=sin[:, sl])
                nc.vector.tensor_mul(out=tcos, in0=ps, in1=cos[:, sl])
                pr = prpool.tile([128, 512], F32, name="pr", tag=prtag, bufs=2)
                mm(pr, lhsT=trid[:, 256:384], rhs=tsn, start=True, stop=True)
                nc.vector.tensor_add(out=dstc[c][:, :], in0=tcos, in1=pr)

            # ---- phase 1: d-interleaved V(st0-3) + K/Q e0 chunks 0,1.
            # Allocation order is deliberate: pv/pk0/pq0 (read early) land in
            # the low banks that the stripe pool will inherit; the chunk-1
            # tiles (read last) land high, away from the early attention work.
            with (
                tc.tile_pool(name="psV1", bufs=1, space="PSUM") as psV1,
                tc.tile_pool(name="psP1", bufs=1, space="PSUM") as psP1,
            ):
                pv = [psV1.tile([128, 512], F32, name=f"pv{i}", tag=f"pv{i}")
                      for i in range(2)]
                pk0 = psP1.tile([128, 512], F32, name="pk0", tag="pk0")
                pq0 = psP1.tile([128, 512], F32, name="pq0", tag="pq0")
                # group A: everything needing only x quarter 0 (cols 0:512)
                for d in range(DT8):
                    for (w_sb, ps) in ((wk_sb, pk0), (wq_sb, pq0)):
                        mm(ps, lhsT=w_sb[:, d * 128:d * 128 + 128],
                           rhs=x_sb[d][:, 0:512],
                           start=(d == 0), stop=(d == DT8 - 1))
                    for st in range(4):
                        mm(pv[st // 2][:, (st % 2) * 256:(st % 2) * 256 + 256],
                           lhsT=x_sb[d][:, st * 128:(st + 1) * 128],
                           rhs=wv_sb[:, d * E:(d + 1) * E],
                           start=(d == 0 and st % 2 == 0),
                           stop=(d == DT8 - 1 and st % 2 == 1))
                with tc.high_priority():
                    rope_chunk(pk0, ktc[0], 0, psP1, "pr")
                    rope_chunk(pq0, qtc[0], 0, psP1, "pr")
                # V st0-3 psum -> v_all, on ACT (idle until first exp)
                for i in range(2):
                    nc.scalar.copy(
                        out=vv[:, 2 * i:2 * i + 2, :, 0:64],
                        in_=pv[i].rearrange("p (s h c) -> p s h c", s=2, h=HPC))
                # group B: chunk-1 projections on x quarter 1
                pk1 = psP1.tile([128, 512], F32, name="pk1", tag="pk1")
                pq1 = psP1.tile([128, 512], F32, name="pq1", tag="pq1")
                for d in range(DT8):
                    for (w_sb, ps) in ((wk_sb, pk1), (wq_sb, pq1)):
                        mm(ps, lhsT=w_sb[:, d * 128:d * 128 + 128],
                           rhs=x_sb[d][:, 512:1024],
                           start=(d == 0), stop=(d == DT8 - 1))
                rope_chunk(pq1, qtc[0], 1, psP1, "pr")
                rope_chunk(pk1, ktc[0], 1, psP1, "pr")

            # ---- attention section. Pool alloc order controls which freed
            # phase-1 banks each pool inherits: stripes first (earliest use).
            psSA = tc.alloc_tile_pool(name="psSA", bufs=1, space="PSUM")
            psA = tc.alloc_tile_pool(name="psA", bufs=1, space="PSUM", side="right")
            sbH = tc.alloc_tile_pool(name="sbH", bufs=1)
            if True:
                # ---- filler step factories (split into half-chains ~1us so
                # they fit the per-kb ACT surplus without starving exps) ----
                def proj_step(w_sb, dstc, e, c, psF):
                    st8 = {}

                    def run1():
                        st8['ps'] = psF.tile([128, 512], F32, name="psF",
                                             tag="psF", bufs=2)
                        for d in range(4):
                            mm(st8['ps'],
                               lhsT=w_sb[:, e * 1024 + d * 128:e * 1024 + d * 128 + 128],
                               rhs=x_sb[d][:, c * 512:(c + 1) * 512],
                               start=(d == 0), stop=False)

                    def run2():
                        ps = st8['ps']
                        for d in range(4, DT8):
                            mm(ps,
                               lhsT=w_sb[:, e * 1024 + d * 128:e * 1024 + d * 128 + 128],
                               rhs=x_sb[d][:, c * 512:(c + 1) * 512],
                               start=False, stop=(d == DT8 - 1))
                        rope_chunk(ps, dstc, c, psF, "psF")

                    return [run1, run2]

                def v_step(st, psF, on_act=False):
                    """V projection for the s-tile pair (st, st+1), st-outer."""
                    st8 = {}

                    def half(d0, d1, first, last):
                        def run():
                            if first:
                                st8['ps'] = psF.tile([128, 512], F32,
                                                     name="psV2", tag="psF",
                                                     bufs=2)
                            ps = st8['ps']
                            for d in range(d0, d1):
                                for i in range(2):
                                    mm(ps[:, i * 256:(i + 1) * 256],
                                       lhsT=x_sb[d][:, (st + i) * 128:(st + i + 1) * 128],
                                       rhs=wv_sb[:, d * E:(d + 1) * E],
                                       start=(first and d == d0 and i == 0),
                                       stop=(last and d == d1 - 1 and i == 1))
                            if last:
                                eng = (nc.scalar.copy if on_act
                                       else nc.vector.tensor_copy)
                                eng(out=vv[:, st:st + 2, :, 0:64],
                                    in_=st8['ps'].rearrange(
                                        "p (s h c) -> p s h c", s=2, h=HPC))
                        return run

                    return [half(0, 4, True, False), half(4, 8, False, True)]

                def batched_norm(h, avq, g4):
                    """normalize 4 q-tiles [g4, g4+4) of head h from avq."""
                    rec = sbH.tile([128, 4], F32, name="rec", tag="rec", bufs=4)
                    av4 = avq[:, 0:260].rearrange("p (j c) -> p j c", c=65)
                    nc.vector.reciprocal(out=rec, in_=av4[:, :, 64:65])
                    nc.vector.tensor_mul(
                        out=attn_t[:, g4:g4 + 4, h * 64:h * 64 + 64],
                        in0=av4[:, :, 0:64],
                        in1=rec.unsqueeze(2).broadcast_to([128, 4, 64]))

                def norm1(h, qt, avq, j):
                    rec = sbH.tile([128, 1], F32, name="rec1", tag="rec1", bufs=4)
                    nc.vector.reciprocal(out=rec,
                                         in_=avq[:, j * 65 + 64:j * 65 + 65])
                    nc.vector.tensor_scalar_mul(
                        out=attn[:, qt * E + h * 64:qt * E + h * 64 + 64],
                        in0=avq[:, j * 65:j * 65 + 64], scalar1=rec[:, 0:1])

                atnT_of = {}

                def tail_t(qt, psM):
                    # transpose attn[qt] (both 128-col halves) -> atnT [e, q]
                    atnT = sbH.tile([128, 256], BF, name="atnT", tag="atnT",
                                    bufs=4)
                    atnT_of[qt] = atnT
                    tp = psM.tile([128, 256], BF, name="tp", tag="tp", bufs=1)
                    for half in range(2):
                        mm(tp[:, half * 128:half * 128 + 128],
                           lhsT=attn[:, qt * E + half * 128:qt * E + half * 128 + 128],
                           rhs=trid[:, 128:256], is_transpose=True,
                           start=(half == 0), stop=(half == 1))
                    nc.vector.tensor_copy(out=atnT, in_=tp)

                def tail_o(qt, psM):
                    atnT = atnT_of.pop(qt)
                    for oc in range(2):
                        po = psM.tile([128, 512], F32, name="po", tag="po",
                                      bufs=2)
                        mm(po, lhsT=atnT[:, 0:128],
                           rhs=wo_sb[:, oc * 512:oc * 512 + 512],
                           start=True, stop=False)
                        mm(po, lhsT=atnT[:, 128:256],
                           rhs=wo_sb[:, D + oc * 512:D + oc * 512 + 512],
                           start=False, stop=True)
                        po_sb = sbH.tile([128, 512], BF, name="posb",
                                         tag="posb", bufs=8)
                        nc.vector.tensor_copy(out=po_sb, in_=po)
                        q_eng = nc.scalar if (qt >= 14 and oc == 1) else nc.sync
                        q_eng.dma_start(
                            out=out_d[qt * 128:(qt + 1) * 128,
                                      oc * 512:(oc + 1) * 512],
                            in_=po_sb)

                def head_A(h, psS, pre=(), filler=(), slots=None):
                    """kb 0-7 x segA [q0, 1024): completes q-tiles 0-7.
                    Emission is staged so all chunk-0 stripes/exps of kb 0-3 go
                    first (they need only Q/K chunk 0 RoPE'd), then chunk 1,
                    then kb 4-7 — the chunk-1 RoPE latency hides behind real
                    exp work instead of blocking the in-order PE queue."""
                    e, hb = h // 2, (h % 2) * 64
                    filler = list(filler)
                    slots = slots or {}
                    if True:
                        avqg = {g4: psA.tile([128, 512], F32, name=f"av{h}a{g4}",
                                             tag="avq", bufs=2) for g4 in (0, 4)}
                        for p in pre:
                            p()

                        def av_mm(kb, qi, pte, base):
                            g4, j = (qi // 4) * 4, qi % 4
                            mm(avqg[g4][:, j * 65:j * 65 + 65],
                               lhsT=pte[:, qi * 128 - base:qi * 128 - base + 128],
                               rhs=v_all[:, kb * 260 + h * 65:kb * 260 + h * 65 + 65],
                               start=(kb == 0 and j == 0),
                               stop=(qi == kb == g4 + 3))

                        ptes = {}

                        def seg(kb, a, b):
                            q0 = kb * 128
                            base = 0 if kb < 4 else 512
                            cb = kb // 4
                            krow = ktc[e][cb][hb:hb + 64,
                                              q0 - cb * 512:q0 - cb * 512 + 128]
                            if kb not in ptes:
                                ptes[kb] = sbH.tile([128, 1024], BF,
                                                    name="pteA", tag="pteA",
                                                    bufs=6)
                            pte = ptes[kb]
                            c = a // 512
                            stripe = psS.tile([128, 512], F32, name="strn",
                                              tag="strn", bufs=3)
                            mm(stripe[:, 0:b - a], lhsT=krow,
                               rhs=qtc[e][c][hb:hb + 64, a - c * 512:b - c * 512],
                               start=True, stop=True)
                            nc.scalar.activation(
                                out=pte[:, a - base:b - base],
                                in_=stripe[:, 0:b - a],
                                func=mybir.ActivationFunctionType.Exp,
                                scale=0.125)
                            if a <= q0 < b:
                                nc.gpsimd.tensor_mul(
                                    out=pte[:, q0 - base:q0 - base + 128],
                                    in0=pte[:, q0 - base:q0 - base + 128],
                                    in1=trid[:, 0:128])

                        pend = []
                        pos = 0

                        def tick():
                            nonlocal pos
                            if pos in slots:
                                filler[slots[pos]]()
                            pos += 1

                        def push(kb):
                            if len(pend) >= 2:
                                pkb = pend.pop(0)
                                for qi in range(pkb, 8):
                                    av_mm(pkb, qi, ptes[pkb],
                                          0 if pkb < 4 else 512)
                            pend.append(kb)

                        for kb in range(4):       # stage A0: chunk 0
                            seg(kb, kb * 128, 512)
                            tick()
                        for kb in range(4):       # stage A1: chunk 1
                            seg(kb, 512, 1024)
                            push(kb)
                            tick()
                        for kb in range(4, 8):    # stage A2: single chunk
                            seg(kb, kb * 128, 1024)
                            push(kb)
                            tick()
                        for pkb in pend:
                            for qi in range(pkb, 8):
                                av_mm(pkb, qi, ptes[pkb], 0 if pkb < 4 else 512)
                        batched_norm(h, avqg[0], 0)
                        batched_norm(h, avqg[4], 4)

                def head_B(h, psS=None, filler=(), slots=None, narrow=False,
                           fuse_out=False, psM=None):
                    """kb 0-15 x segB [max(1024,q0), 2048): q-tiles 8-15."""
                    e, hb = h // 2, (h % 2) * 64
                    filler = list(filler)
                    slots = slots or {}
                    own = psS is None
                    if own:
                        psS = tc.alloc_tile_pool(name=f"psS{h}b", bufs=1,
                                                 space="PSUM")
                    if True:
                        avqg = {g4: psA.tile([128, 512], F32, name=f"av{h}b{g4}",
                                             tag="avq", bufs=2) for g4 in (8, 12)}
                        pendT = []

                        def av_mm(kb, qi, pte, base):
                            g4, j = (qi // 4) * 4, qi % 4
                            mm(avqg[g4][:, j * 65:j * 65 + 65],
                               lhsT=pte[:, qi * 128 - base:qi * 128 - base + 128],
                               rhs=v_all[:, kb * 260 + h * 65:kb * 260 + h * 65 + 65],
                               start=(kb == 0 and j == 0),
                               stop=(qi == kb == g4 + 3))
                            if fuse_out and qi == kb:
                                norm1(h, kb, avqg[g4], j)
                                tail_t(kb, psM)
                                pendT.append(kb)
                                if len(pendT) >= 2:
                                    tail_o(pendT.pop(0), psM)

                        pend = []
                        for kb in range(ST):
                            q0 = kb * 128
                            sa = 1024 if kb < 8 else q0
                            base = 1024 if kb < 8 else (q0 // 512) * 512
                            cb = kb // 4
                            krow = ktc[e][cb][hb:hb + 64,
                                              q0 - cb * 512:q0 - cb * 512 + 128]
                            pte = sbH.tile([128, 1024], BF, name="pteB",
                                           tag="pteB", bufs=4)
                            if narrow:
                                for (a, b) in _chunks(sa, 2048):
                                    c = a // 512
                                    stripe = psS.tile([128, 512], F32,
                                                      name="strn", tag="strn",
                                                      bufs=3)
                                    mm(stripe[:, 0:b - a], lhsT=krow,
                                       rhs=qtc[e][c][hb:hb + 64,
                                                     a - c * 512:b - c * 512],
                                       start=True, stop=True)
                                    nc.scalar.activation(
                                        out=pte[:, a - base:b - base],
                                        in_=stripe[:, 0:b - a],
                                        func=mybir.ActivationFunctionType.Exp,
                                        scale=0.125)
                            else:
                                stripe = psS.tile([128, 1024], F32,
                                                  name="strw", tag="strw",
                                                  bufs=2)
                                for (a, b) in _chunks(sa, 2048):
                                    c = a // 512
                                    mm(stripe[:, a - base:b - base], lhsT=krow,
                                       rhs=qtc[e][c][hb:hb + 64,
                                                     a - c * 512:b - c * 512],
                                       start=True, stop=True)
                                nc.scalar.activation(
                                    out=pte[:, sa - base:2048 - base],
                                    in_=stripe[:, sa - base:2048 - base],
                                    func=mybir.ActivationFunctionType.Exp,
                                    scale=0.125)
                            if kb >= 8:
                                nc.gpsimd.tensor_mul(
                                    out=pte[:, q0 - base:q0 - base + 128],
                                    in0=pte[:, q0 - base:q0 - base + 128],
                                    in1=trid[:, 0:128])
                            lag = 1 if (fuse_out and kb >= 13) else 2
                            while pend and pend[0][0] <= kb - lag:
                                pkb, ppte, pbase = pend.pop(0)
                                for qi in range(max(8, pkb), ST):
                                    av_mm(pkb, qi, ppte, pbase)
                            if kb in slots:
                                filler[slots[kb]]()
                            pend.append((kb, pte, base))
                        for pkb, ppte, pbase in pend:
                            for qi in range(max(8, pkb), ST):
                                av_mm(pkb, qi, ppte, pbase)
                        if fuse_out:
                            for qt in pendT:
                                tail_o(qt, psM)
                        else:
                            batched_norm(h, avqg[8], 8)
                            batched_norm(h, avqg[12], 12)
                    if own:
                        psS.release()

                # ---- pass schedule ----
                psF = tc.alloc_tile_pool(name="psF", bufs=1, space="PSUM", side="right")
                head_A(0, psSA,
                       filler=v_step(4, psF, on_act=True) + v_step(6, psF),
                       slots={4: 0, 6: 1, 8: 2, 10: 3})
                head_A(1, psSA, filler=proj_step(wq_sb, qtc[1], 1, 0, psF)
                                       + proj_step(wk_sb, ktc[1], 1, 0, psF)
                                       + proj_step(wq_sb, qtc[1], 1, 1, psF)
                                       + proj_step(wk_sb, ktc[1], 1, 1, psF),
                       slots={1: 0, 2: 1, 4: 2, 5: 3, 7: 4, 8: 5,
                              10: 6, 11: 7})
                head_A(2, psSA, filler=proj_step(wq_sb, qtc[0], 0, 2, psF)
                                       + proj_step(wq_sb, qtc[0], 0, 3, psF),
                       slots={2: 0, 5: 1, 8: 2, 11: 3})
                head_A(3, psSA, filler=v_step(8, psF) + v_step(10, psF)
                                       + v_step(12, psF) + v_step(14, psF),
                       slots={1: 0, 2: 1, 4: 2, 5: 3, 7: 4, 8: 5,
                              10: 6, 11: 7})
                psSA.release()
                head_B(0, filler=proj_step(wk_sb, ktc[0], 0, 2, psF)
                                 + proj_step(wk_sb, ktc[0], 0, 3, psF)
                                 + proj_step(wq_sb, qtc[1], 1, 2, psF)
                                 + proj_step(wq_sb, qtc[1], 1, 3, psF),
                       slots={i: i for i in range(8)})
                head_B(1, filler=proj_step(wk_sb, ktc[1], 1, 2, psF)
                                 + proj_step(wk_sb, ktc[1], 1, 3, psF),
                       slots={1: 0, 3: 1, 5: 2, 7: 3})
                psF.release()
                psSB = tc.alloc_tile_pool(name="psSB", bufs=1, space="PSUM")
                psM = tc.alloc_tile_pool(name="psM", bufs=1, space="PSUM", side="right")
                wo_units = []
                for qt in range(8):
                    wo_units.append(lambda qt=qt: tail_t(qt, psM))
                    wo_units.append(lambda qt=qt: tail_o(qt, psM))
                # interleave t/o one-lagged: t0,t1,o0,t2,o1,...,t7,o6,o7
                seq = [wo_units[0], wo_units[2]]
                for i in range(6):
                    seq += [wo_units[2 * i + 1], wo_units[2 * i + 4]]
                seq += [wo_units[13], wo_units[15]]
                head_B(2, psSB, filler=seq[0:8], narrow=True,
                       slots={1: 0, 3: 1, 5: 2, 7: 3, 9: 4, 11: 5,
                              13: 6, 14: 7}, psM=psM)
                head_B(3, psSB, filler=seq[8:16], narrow=True,
                       slots={i: i for i in range(8)},
                       fuse_out=True, psM=psM)
                psSB.release()
                psM.release()
                psA.release()
                sbH.release()

    nc.compile()
    return nc


def _host_inputs(x, token_positions, Wq, Wk, Wv, Wo):
    import ml_dtypes
    bf = ml_dtypes.bfloat16

    def pack(a, nblk):
        # [nblk*128, w] -> [128, nblk*w] with block d at cols [d*w, (d+1)*w)
        w = a.shape[1]
        return np.ascontiguousarray(
            a.reshape(nblk, 128, w).transpose(1, 0, 2).reshape(128, nblk * w))

    def pack_emajor(a):
        # a: [1024 (d rows), 256 (e cols)] -> [128, e*1024 + d*128 + j]
        return np.ascontiguousarray(
            a.reshape(8, 128, 2, 128).transpose(1, 2, 0, 3).reshape(128, 2048))

    perm = np.concatenate([np.arange(0, DK, 2), np.arange(1, DK, 2)])
    inv_freq = THETA ** (-np.arange(0, DK, 2, dtype=np.float64) / DK)
    swap32 = np.zeros((128, 128), np.float32)
    swap32[np.arange(128) ^ 32, np.arange(128)] = 1.0
    trid = np.concatenate(
        [np.triu(np.ones((128, 128), np.float32)),
         np.eye(128, dtype=np.float32), swap32],
        axis=1)
    in_maps = []
    for c in range(NCORES):
        b, g = divmod(c, NCORES // B)
        heads = [(g * HPC + h) for h in range(HPC)]
        rows_rope = np.concatenate([h * DK + perm for h in heads])
        rows_plain = np.concatenate([h * DK + np.arange(DK) for h in heads])
        pos = token_positions[b].astype(np.float64)
        ang = inv_freq[:, None] * pos[None, :]  # [32, S]
        cosv = np.cos(ang).astype(np.float32)
        sinv = np.sin(ang).astype(np.float32)
        cosF = np.concatenate([cosv] * 4, axis=0)  # [128, S]
        sinF = np.concatenate([sinv, -sinv, sinv, -sinv], axis=0)
        in_maps.append({
            "x_d": pack(np.ascontiguousarray(x[b].T), DT8).astype(bf),
            "wq_d": pack_emajor(np.ascontiguousarray(Wq[rows_rope, :].T)).astype(bf),
            "wk_d": pack_emajor(np.ascontiguousarray(Wk[rows_rope, :].T)).astype(bf),
            "wv_d": pack(np.ascontiguousarray(Wv[rows_plain, :].T), DT8).astype(bf),
            "wo_d": pack(np.ascontiguousarray(Wo[:, rows_plain].T), 2).astype(bf),
            "cos_d": cosF.astype(bf),
            "sin_d": sinF.astype(bf),
            "trid_d": trid.astype(bf),
        })
    return in_maps


def kernel(x, token_positions, Wq, Wk, Wv, Wo, _debug=False):
    x = np.asarray(x, np.float32)
    token_positions = np.asarray(token_positions, np.int32)
    Wq, Wk, Wv, Wo = (np.asarray(w, np.float32) for w in (Wq, Wk, Wv, Wo))
    if "nc" not in _cache:
        _cache["nc"] = _build_nc()
    nc = _cache["nc"]
    in_maps = _host_inputs(x, token_positions, Wq, Wk, Wv, Wo)
    res = run_bass_kernel_spmd(
        nc, in_maps, core_ids=list(range(NCORES)), trace=False)
    outs = [np.asarray(r["out"], np.float32) for r in res.results]
    full = np.zeros((B, S, D), np.float32)
    for c in range(NCORES):
        full[c // (NCORES // B)] += outs[c]
    if _debug:
        return full, res
    return full


# revision 34
# speedup vs baseline: 1.0107x; 1.0107x over previous
"""Multi-head self-attention with RoPE on 8 Trainium2 NeuronCores.

Sharding: data-parallel over batch (2) x tensor-parallel over heads (4 groups
of 4 heads). Each core computes its heads' attention plus a partial output
projection (row-sharded Wo); the host sums the 4 partials per batch.

v2 design (bf16 matmul inputs, f32 PSUM accumulation). The kernel is PE-bound
(~105us of matmul column-time at 0.42ns/col); everything else is scheduled to
keep PE busy end to end and to start the ACT exp stream as early as possible:
  - Phase 1 interleaves, per x d-tile as its first half arrives from HBM:
    V-proj for s-tiles 0-3 + the Q/K e0-block projections for chunks 0,1
    (RoPE per chunk, swap DMAs on the DVE queue). First exp fires ~15us.
  - Each head's attention is split into an A-pass (kb 0-7 x q in [q0,1024):
    complete attention for q-tiles 0-7, needs only chunks 0/1 of Q/K e0|e1)
    and a B-pass (kb 0-15 x q in [max(1024,q0), 2048): q-tiles 8-15).
    Pass order h0A..h3A, h0B..h3B; remaining projection chunks, V s-tiles
    4-15 (st-outer) and the Wo tails ride as fillers in the exp gaps.
  - Scores transposed PT[k,q] per kb; exp on ACT; AV natural out[q,65] with a
    ones-column in V giving the softmax denominator per q in-partition.
    Causal diag masks run on the idle GPSIMD engine.
  - Normalization batched per pass: one reciprocal [128,4] + one broadcast
    tensor-mul per 4 q-tiles; progressive per-tile in h3B to fuse Wo+store.
  - attn[q,e] PE-transposed per q-tile into atnT for the natural-layout Wo;
    bf16 partial stores (host sums 4 partials per batch).
  - PSUM budget: A-passes 3 stripe banks + 2 avq + 2 filler; B wide passes
    4 stripe + 2 avq + 2 filler; h2B/h3B 3 stripe + 2 avq + 3 tail (tp+po).
"""
import os
import sys

import numpy as np

for _p in ("/opt/trn_rl_repo", "/root/.axon_site/_ro/trn_rl_repo"):
    if os.path.isdir(_p) and _p not in sys.path:
        sys.path.insert(0, _p)
        break

import concourse.bacc as bacc
import concourse.tile as tile
from concourse import mybir
from concourse.bass_utils import run_bass_kernel_spmd

B, S, D, H = 2, 2048, 1024, 16
DK = 64
THETA = 10000.0
NCORES = 8
HPC = H // (NCORES // B)  # heads per core = 4
E = HPC * DK              # local dims per core = 256
DT8 = D // 128            # 8 d-tiles
ST = S // 128             # 16 s-tiles
CH = S // 512             # 4 512-chunks
F32 = mybir.dt.float32
BF = mybir.dt.bfloat16

_cache = {}


def _chunks(a, b):
    """Split [a, b) at absolute multiples of 512 (PSUM bank boundaries)."""
    out = []
    c0 = a
    while c0 < b:
        c1 = min(b, (c0 // 512 + 1) * 512)
        out.append((c0, c1))
        c0 = c1
    return out


def _build_nc():
    nc = bacc.Bacc(
        "TRN2",
        target_bir_lowering=False,
        debug=False,
        enable_asserts=False,
        num_devices=NCORES,
    )

    def mm(out, lhsT, rhs, **kw):
        nc.tensor.matmul(out, lhsT, rhs, **kw)

    # ---- I/O (all packed on host into SBUF-ready [128, *] layouts) ----
    x_d = nc.dram_tensor("x_d", [128, DT8 * S], BF, kind="ExternalInput").ap()
    # wq/wk packed e-major: cols [e*1024 + d*128 + j]
    wq_d = nc.dram_tensor("wq_d", [128, 2 * 1024], BF, kind="ExternalInput").ap()
    wk_d = nc.dram_tensor("wk_d", [128, 2 * 1024], BF, kind="ExternalInput").ap()
    wv_d = nc.dram_tensor("wv_d", [128, DT8 * E], BF, kind="ExternalInput").ap()
    wo_d = nc.dram_tensor("wo_d", [128, 2 * D], BF, kind="ExternalInput").ap()
    cos_d = nc.dram_tensor("cos_d", [128, S], BF, kind="ExternalInput").ap()
    sin_d = nc.dram_tensor("sin_d", [128, S], BF, kind="ExternalInput").ap()
    trid_d = nc.dram_tensor("trid_d", [128, 384], BF, kind="ExternalInput").ap()
    out_d = nc.dram_tensor("out", [S, D], BF, kind="ExternalOutput").ap()

    with tile.TileContext(nc) as tc:
      with (
          tc.tile_pool(name="const", bufs=1) as cp,
          tc.tile_pool(name="pers", bufs=1) as pp,
      ):
        cos = cp.tile([128, S], BF, name="cos", tag="cos")
        sin = cp.tile([128, S], BF, name="sin", tag="sin")
        trid = cp.tile([128, 384], BF, name="trid", tag="trid")
        wq_sb = pp.tile([128, 2 * 1024], BF, name="wq", tag="wq")
        wk_sb = pp.tile([128, 2 * 1024], BF, name="wk", tag="wk")
        wv_sb = pp.tile([128, DT8 * E], BF, name="wv", tag="wv")
        wo_sb = pp.tile([128, 2 * D], BF, name="wo", tag="wo")
        x_sb = [pp.tile([128, S], BF, name=f"x{d}", tag=f"x{d}") for d in range(DT8)]
        # per-512-chunk q/k tiles so scores only wait on the chunks they read
        qtc = [[pp.tile([128, 512], BF, name=f"qt{e}c{c}", tag=f"qt{e}c{c}")
                for c in range(CH)] for e in range(2)]
        ktc = [[pp.tile([128, 512], BF, name=f"kt{e}c{c}", tag=f"kt{e}c{c}")
                for c in range(CH)] for e in range(2)]
        v_all = pp.tile([128, ST * (E + HPC)], BF, name="vall", tag="vall")
        attn = pp.tile([128, ST * E], BF, name="attn", tag="attn")

        # ---- input loads, ALL on the SP queue: dma_start holds the issuing
        # queue's SEQ until the shared HWDGE grants it, so putting any input
        # load on the ACT queue would block the exp stream behind the whole
        # input sequence. SP has nothing else to do until the po stores.
        ld = nc.sync.dma_start
        ld(out=wk_sb[:, 0:1024], in_=wk_d[:, 0:1024])
        ld(out=wq_sb[:, 0:1024], in_=wq_d[:, 0:1024])
        ld(out=wv_sb[:, 0:512], in_=wv_d[:, 0:512])
        ld(out=x_sb[0][:, 0:512], in_=x_d[:, 0:512])
        ld(out=wv_sb[:, 512:2048], in_=wv_d[:, 512:2048])
        for d in range(1, DT8):
            ld(out=x_sb[d][:, 0:512], in_=x_d[:, d * S:d * S + 512])
        for d in range(0, 4):
            ld(out=x_sb[d][:, 512:1024], in_=x_d[:, d * S + 512:d * S + 1024])
        ld(out=cos[:, 0:512], in_=cos_d[:, 0:512])
        ld(out=sin[:, 0:512], in_=sin_d[:, 0:512])
        for d in range(4, DT8):
            ld(out=x_sb[d][:, 512:1024], in_=x_d[:, d * S + 512:d * S + 1024])
        ld(out=cos[:, 512:1024], in_=cos_d[:, 512:1024])
        ld(out=sin[:, 512:1024], in_=sin_d[:, 512:1024])
        ld(out=trid, in_=trid_d)
        for d in range(DT8):
            ld(out=x_sb[d][:, 1024:2048], in_=x_d[:, d * S + 1024:(d + 1) * S])
        ld(out=cos[:, 1024:2048], in_=cos_d[:, 1024:2048])
        ld(out=sin[:, 1024:2048], in_=sin_d[:, 1024:2048])
        ld(out=wk_sb[:, 1024:2048], in_=wk_d[:, 1024:2048])
        ld(out=wq_sb[:, 1024:2048], in_=wq_d[:, 1024:2048])
        ld(out=wo_sb, in_=wo_d)

        # ones columns for the softmax denominators
        nc.gpsimd.memset(
            v_all.rearrange("p (st h c) -> p st h c", st=ST, h=HPC)[:, :, :, 64:65],
            1.0)

        vv = v_all.rearrange("p (st h c) -> p st h c", st=ST, h=HPC)
        attn_t = attn.rearrange("p (t e) -> p t e", t=ST)

        with tc.tile_pool(name="rope", bufs=1) as rp:

            def rope_chunk(ps, dstc, c, prpool, prtag):
                """Finish RoPE for one 512-chunk from its raw psum projection:
                dst = ps*cos + rowswap32(ps*sin'). The 32-row-group swap is a
                PE matmul by a constant permutation matrix (trid cols 256:384)
                — no DMA, no HWDGE, no DMA-semaphore latency. tsin/tcos are
                per-call rotating tiles so chunks pipeline independently."""
                sl = slice(c * 512, (c + 1) * 512)
                tsn = rp.tile([128, 512], BF, name="tsn", tag="tsn", bufs=4)
                tcos = rp.tile([128, 512], BF, name="tcos", tag="tcos", bufs=4)
                nc.vector.tensor_mul(out=tsn, in0=ps, in1=sin[:, sl])
                nc.vector.tensor_mul(out=tcos, in0=ps, in1=cos[:, sl])
                pr = prpool.tile([128, 512], F32, name="pr", tag=prtag, bufs=2)
                mm(pr, lhsT=trid[:, 256:384], rhs=tsn, start=True, stop=True)
                nc.vector.tensor_add(out=dstc[c][:, :], in0=tcos, in1=pr)

            # ---- phase 1: d-interleaved V(st0-3) + K/Q e0 chunks 0,1.
            # Allocation order is deliberate: pv/pk0/pq0 (read early) land in
            # the low banks that the stripe pool will inherit; the chunk-1
            # tiles (read last) land high, away from the early attention work.
            with (
                tc.tile_pool(name="psV1", bufs=1, space="PSUM") as psV1,
                tc.tile_pool(name="psP1", bufs=1, space="PSUM") as psP1,
            ):
                pv = [psV1.tile([128, 512], F32, name=f"pv{i}", tag=f"pv{i}")
                      for i in range(2)]
                pk0 = psP1.tile([128, 512], F32, name="pk0", tag="pk0")
                pq0 = psP1.tile([128, 512], F32, name="pq0", tag="pq0")
                # group A: everything needing only x quarter 0 (cols 0:512)
                for d in range(DT8):
                    for (w_sb, ps) in ((wk_sb, pk0), (wq_sb, pq0)):
                        mm(ps, lhsT=w_sb[:, d * 128:d * 128 + 128],
                           rhs=x_sb[d][:, 0:512],
                           start=(d == 0), stop=(d == DT8 - 1))
                    for st in range(4):
                        mm(pv[st // 2][:, (st % 2) * 256:(st % 2) * 256 + 256],
                           lhsT=x_sb[d][:, st * 128:(st + 1) * 128],
                           rhs=wv_sb[:, d * E:(d + 1) * E],
                           start=(d == 0 and st % 2 == 0),
                           stop=(d == DT8 - 1 and st % 2 == 1))
                with tc.high_priority():
                    rope_chunk(pk0, ktc[0], 0, psP1, "pr")
                    rope_chunk(pq0, qtc[0], 0, psP1, "pr")
                # V st0-3 psum -> v_all, on ACT (idle until first exp)
                for i in range(2):
                    nc.scalar.copy(
                        out=vv[:, 2 * i:2 * i + 2, :, 0:64],
                        in_=pv[i].rearrange("p (s h c) -> p s h c", s=2, h=HPC))
                # group B: chunk-1 projections on x quarter 1
                pk1 = psP1.tile([128, 512], F32, name="pk1", tag="pk1")
                pq1 = psP1.tile([128, 512], F32, name="pq1", tag="pq1")
                for d in range(DT8):
                    for (w_sb, ps) in ((wk_sb, pk1), (wq_sb, pq1)):
                        mm(ps, lhsT=w_sb[:, d * 128:d * 128 + 128],
                           rhs=x_sb[d][:, 512:1024],
                           start=(d == 0), stop=(d == DT8 - 1))
                rope_chunk(pq1, qtc[0], 1, psP1, "pr")
                rope_chunk(pk1, ktc[0], 1, psP1, "pr")

            # ---- attention section. Pool alloc order controls which freed
            # phase-1 banks each pool inherits: stripes first (earliest use).
            psSA = tc.alloc_tile_pool(name="psSA", bufs=1, space="PSUM")
            psA = tc.alloc_tile_pool(name="psA", bufs=1, space="PSUM", side="right")
            sbH = tc.alloc_tile_pool(name="sbH", bufs=1)
            if True:
                # ---- filler step factories (split into half-chains ~1us so
                # they fit the per-kb ACT surplus without starving exps) ----
                def proj_step(w_sb, dstc, e, c, psF):
                    st8 = {}

                    def run1():
                        st8['ps'] = psF.tile([128, 512], F32, name="psF",
                                             tag="psF", bufs=2)
                        for d in range(4):
                            mm(st8['ps'],
                               lhsT=w_sb[:, e * 1024 + d * 128:e * 1024 + d * 128 + 128],
                               rhs=x_sb[d][:, c * 512:(c + 1) * 512],
                               start=(d == 0), stop=False)

                    def run2():
                        ps = st8['ps']
                        for d in range(4, DT8):
                            mm(ps,
                               lhsT=w_sb[:, e * 1024 + d * 128:e * 1024 + d * 128 + 128],
                               rhs=x_sb[d][:, c * 512:(c + 1) * 512],
                               start=False, stop=(d == DT8 - 1))
                        rope_chunk(ps, dstc, c, psF, "psF")

                    return [run1, run2]

                def v_step(st, psF, on_act=False):
                    """V projection for the s-tile pair (st, st+1), st-outer."""
                    st8 = {}

                    def half(d0, d1, first, last):
                        def run():
                            if first:
                                st8['ps'] = psF.tile([128, 512], F32,
                                                     name="psV2", tag="psF",
                                                     bufs=2)
                            ps = st8['ps']
                            for d in range(d0, d1):
                                for i in range(2):
                                    mm(ps[:, i * 256:(i + 1) * 256],
                                       lhsT=x_sb[d][:, (st + i) * 128:(st + i + 1) * 128],
                                       rhs=wv_sb[:, d * E:(d + 1) * E],
                                       start=(first and d == d0 and i == 0),
                                       stop=(last and d == d1 - 1 and i == 1))
                            if last:
                                eng = (nc.scalar.copy if on_act
                                       else nc.vector.tensor_copy)
                                eng(out=vv[:, st:st + 2, :, 0:64],
                                    in_=st8['ps'].rearrange(
                                        "p (s h c) -> p s h c", s=2, h=HPC))
                        return run

                    return [half(0, 4, True, False), half(4, 8, False, True)]

                def batched_norm(h, avq, g4):
                    """normalize 4 q-tiles [g4, g4+4) of head h from avq."""
                    rec = sbH.tile([128, 4], F32, name="rec", tag="rec", bufs=4)
                    av4 = avq[:, 0:260].rearrange("p (j c) -> p j c", c=65)
                    nc.vector.reciprocal(out=rec, in_=av4[:, :, 64:65])
                    nc.vector.tensor_mul(
                        out=attn_t[:, g4:g4 + 4, h * 64:h * 64 + 64],
                        in0=av4[:, :, 0:64],
                        in1=rec.unsqueeze(2).broadcast_to([128, 4, 64]))

                def norm1(h, qt, avq, j):
                    rec = sbH.tile([128, 1], F32, name="rec1", tag="rec1", bufs=4)
                    nc.vector.reciprocal(out=rec,
                                         in_=avq[:, j * 65 + 64:j * 65 + 65])
                    nc.vector.tensor_scalar_mul(
                        out=attn[:, qt * E + h * 64:qt * E + h * 64 + 64],
                        in0=avq[:, j * 65:j * 65 + 64], scalar1=rec[:, 0:1])

                atnT_of = {}

                def tail_t(qt, psM):
                    # transpose attn[qt] (both 128-col halves) -> atnT [e, q]
                    atnT = sbH.tile([128, 256], BF, name="atnT", tag="atnT",
                                    bufs=4)
                    atnT_of[qt] = atnT
                    tp = psM.tile([128, 256], BF, name="tp", tag="tp", bufs=1)
                    for half in range(2):
                        mm(tp[:, half * 128:half * 128 + 128],
                           lhsT=attn[:, qt * E + half * 128:qt * E + half * 128 + 128],
                           rhs=trid[:, 128:256], is_transpose=True,
                           start=(half == 0), stop=(half == 1))
                    nc.vector.tensor_copy(out=atnT, in_=tp)

                def tail_o(qt, psM):
                    atnT = atnT_of.pop(qt)
                    for oc in range(2):
                        po = psM.tile([128, 512], F32, name="po", tag="po",
                                      bufs=2)
                        mm(po, lhsT=atnT[:, 0:128],
                           rhs=wo_sb[:, oc * 512:oc * 512 + 512],
                           start=True, stop=False)
                        mm(po, lhsT=atnT[:, 128:256],
                           rhs=wo_sb[:, D + oc * 512:D + oc * 512 + 512],
                           start=False, stop=True)
                        po_sb = sbH.tile([128, 512], BF, name="posb",
                                         tag="posb", bufs=8)
                        # late tiles: copy on ACT (idle once exps are done)
                        # so the tail isn't serialized on DVE
                        if qt >= 10:
                            nc.scalar.copy(out=po_sb, in_=po)
                        else:
                            nc.vector.tensor_copy(out=po_sb, in_=po)
                        q_eng = nc.scalar if qt >= 14 else nc.sync
                        q_eng.dma_start(
                            out=out_d[qt * 128:(qt + 1) * 128,
                                      oc * 512:(oc + 1) * 512],
                            in_=po_sb)

                def head_A(h, psS, pre=(), filler=(), slots=None):
                    """kb 0-7 x segA [q0, 1024): completes q-tiles 0-7.
                    Emission is staged so all chunk-0 stripes/exps of kb 0-3 go
                    first (they need only Q/K chunk 0 RoPE'd), then chunk 1,
                    then kb 4-7 — the chunk-1 RoPE latency hides behind real
                    exp work instead of blocking the in-order PE queue."""
                    e, hb = h // 2, (h % 2) * 64
                    filler = list(filler)
                    slots = slots or {}
                    if True:
                        avqg = {g4: psA.tile([128, 512], F32, name=f"av{h}a{g4}",
                                             tag="avq", bufs=2) for g4 in (0, 4)}
                        for p in pre:
                            p()

                        def av_mm(kb, qi, pte, base):
                            g4, j = (qi // 4) * 4, qi % 4
                            mm(avqg[g4][:, j * 65:j * 65 + 65],
                               lhsT=pte[:, qi * 128 - base:qi * 128 - base + 128],
                               rhs=v_all[:, kb * 260 + h * 65:kb * 260 + h * 65 + 65],
                               start=(kb == 0 and j == 0),
                               stop=(qi == kb == g4 + 3))

                        ptes = {}

                        def seg(kb, a, b):
                            q0 = kb * 128
                            base = 0 if kb < 4 else 512
                            cb = kb // 4
                            krow = ktc[e][cb][hb:hb + 64,
                                              q0 - cb * 512:q0 - cb * 512 + 128]
                            if kb not in ptes:
                                ptes[kb] = sbH.tile([128, 1024], BF,
                                                    name="pteA", tag="pteA",
                                                    bufs=6)
                            pte = ptes[kb]
                            c = a // 512
                            stripe = psS.tile([128, 512], F32, name="strn",
                                              tag="strn", bufs=3)
                            mm(stripe[:, 0:b - a], lhsT=krow,
                               rhs=qtc[e][c][hb:hb + 64, a - c * 512:b - c * 512],
                               start=True, stop=True)
                            nc.scalar.activation(
                                out=pte[:, a - base:b - base],
                                in_=stripe[:, 0:b - a],
                                func=mybir.ActivationFunctionType.Exp,
                                scale=0.125)
                            if a <= q0 < b:
                                nc.gpsimd.tensor_mul(
                                    out=pte[:, q0 - base:q0 - base + 128],
                                    in0=pte[:, q0 - base:q0 - base + 128],
                                    in1=trid[:, 0:128])

                        pend = []
                        pos = 0

                        def tick():
                            nonlocal pos
                            if pos in slots:
                                filler[slots[pos]]()
                            pos += 1

                        def push(kb):
                            if len(pend) >= 2:
                                pkb = pend.pop(0)
                                for qi in range(pkb, 8):
                                    av_mm(pkb, qi, ptes[pkb],
                                          0 if pkb < 4 else 512)
                            pend.append(kb)

                        for kb in range(4):       # stage A0: chunk 0
                            seg(kb, kb * 128, 512)
                            tick()
                        for kb in range(4):       # stage A1: chunk 1
                            seg(kb, 512, 1024)
                            push(kb)
                            tick()
                        for kb in range(4, 8):    # stage A2: single chunk
                            seg(kb, kb * 128, 1024)
                            push(kb)
                            tick()
                        for pkb in pend:
                            for qi in range(pkb, 8):
                                av_mm(pkb, qi, ptes[pkb], 0 if pkb < 4 else 512)
                        batched_norm(h, avqg[0], 0)
                        batched_norm(h, avqg[4], 4)

                def head_B(h, psS=None, filler=(), slots=None, narrow=False,
                           fuse_out=False, psM=None):
                    """kb 0-15 x segB [max(1024,q0), 2048): q-tiles 8-15."""
                    e, hb = h // 2, (h % 2) * 64
                    filler = list(filler)
                    slots = slots or {}
                    own = psS is None
                    if own:
                        psS = tc.alloc_tile_pool(name=f"psS{h}b", bufs=1,
                                                 space="PSUM")
                    if True:
                        avqg = {g4: psA.tile([128, 512], F32, name=f"av{h}b{g4}",
                                             tag="avq", bufs=2) for g4 in (8, 12)}
                        pendT = []

                        def av_mm(kb, qi, pte, base):
                            g4, j = (qi // 4) * 4, qi % 4
                            mm(avqg[g4][:, j * 65:j * 65 + 65],
                               lhsT=pte[:, qi * 128 - base:qi * 128 - base + 128],
                               rhs=v_all[:, kb * 260 + h * 65:kb * 260 + h * 65 + 65],
                               start=(kb == 0 and j == 0),
                               stop=(qi == kb == g4 + 3))
                            if fuse_out and qi == kb:
                                norm1(h, kb, avqg[g4], j)
                                tail_t(kb, psM)
                                pendT.append(kb)
                                if len(pendT) >= 2:
                                    tail_o(pendT.pop(0), psM)

                        pend = []
                        for kb in range(ST):
                            q0 = kb * 128
                            sa = 1024 if kb < 8 else q0
                            base = 1024 if kb < 8 else (q0 // 512) * 512
                            cb = kb // 4
                            krow = ktc[e][cb][hb:hb + 64,
                                              q0 - cb * 512:q0 - cb * 512 + 128]
                            pte = sbH.tile([128, 1024], BF, name="pteB",
                                           tag="pteB", bufs=4)
                            if narrow:
                                for (a, b) in _chunks(sa, 2048):
                                    c = a // 512
                                    stripe = psS.tile([128, 512], F32,
                                                      name="strn", tag="strn",
                                                      bufs=3)
                                    mm(stripe[:, 0:b - a], lhsT=krow,
                                       rhs=qtc[e][c][hb:hb + 64,
                                                     a - c * 512:b - c * 512],
                                       start=True, stop=True)
                                    nc.scalar.activation(
                                        out=pte[:, a - base:b - base],
                                        in_=stripe[:, 0:b - a],
                                        func=mybir.ActivationFunctionType.Exp,
                                        scale=0.125)
                            else:
                                stripe = psS.tile([128, 1024], F32,
                                                  name="strw", tag="strw",
                                                  bufs=2)
                                for (a, b) in _chunks(sa, 2048):
                                    c = a // 512
                                    mm(stripe[:, a - base:b - base], lhsT=krow,
                                       rhs=qtc[e][c][hb:hb + 64,
                                                     a - c * 512:b - c * 512],
                                       start=True, stop=True)
                                nc.scalar.activation(
                                    out=pte[:, sa - base:2048 - base],
                                    in_=stripe[:, sa - base:2048 - base],
                                    func=mybir.ActivationFunctionType.Exp,
                                    scale=0.125)
                            if kb >= 8:
                                nc.gpsimd.tensor_mul(
                                    out=pte[:, q0 - base:q0 - base + 128],
                                    in0=pte[:, q0 - base:q0 - base + 128],
                                    in1=trid[:, 0:128])
                            lag = 1 if (fuse_out and kb >= 13) else 2
                            while pend and pend[0][0] <= kb - lag:
                                pkb, ppte, pbase = pend.pop(0)
                                for qi in range(max(8, pkb), ST):
                                    av_mm(pkb, qi, ppte, pbase)
                            if kb in slots:
                                filler[slots[kb]]()
                            pend.append((kb, pte, base))
                        for pkb, ppte, pbase in pend:
                            for qi in range(max(8, pkb), ST):
                                av_mm(pkb, qi, ppte, pbase)
                        if fuse_out:
                            for qt in pendT:
                                tail_o(qt, psM)
                        else:
                            batched_norm(h, avqg[8], 8)
                            batched_norm(h, avqg[12], 12)
                    if own:
                        psS.release()

                # ---- pass schedule ----
                psF = tc.alloc_tile_pool(name="psF", bufs=1, space="PSUM", side="right")
                head_A(0, psSA,
                       filler=v_step(4, psF, on_act=True) + v_step(6, psF),
                       slots={4: 0, 6: 1, 8: 2, 10: 3})
                head_A(1, psSA, filler=proj_step(wq_sb, qtc[1], 1, 0, psF)
                                       + proj_step(wk_sb, ktc[1], 1, 0, psF)
                                       + proj_step(wq_sb, qtc[1], 1, 1, psF)
                                       + proj_step(wk_sb, ktc[1], 1, 1, psF),
                       slots={1: 0, 2: 1, 4: 2, 5: 3, 7: 4, 8: 5,
                              10: 6, 11: 7})
                head_A(2, psSA, filler=proj_step(wq_sb, qtc[0], 0, 2, psF)
                                       + proj_step(wq_sb, qtc[0], 0, 3, psF),
                       slots={2: 0, 5: 1, 8: 2, 11: 3})
                head_A(3, psSA, filler=v_step(8, psF) + v_step(10, psF)
                                       + v_step(12, psF) + v_step(14, psF),
                       slots={1: 0, 2: 1, 4: 2, 5: 3, 7: 4, 8: 5,
                              10: 6, 11: 7})
                psSA.release()
                head_B(0, filler=proj_step(wk_sb, ktc[0], 0, 2, psF)
                                 + proj_step(wk_sb, ktc[0], 0, 3, psF)
                                 + proj_step(wq_sb, qtc[1], 1, 2, psF)
                                 + proj_step(wq_sb, qtc[1], 1, 3, psF),
                       slots={i: i for i in range(8)})
                head_B(1, filler=proj_step(wk_sb, ktc[1], 1, 2, psF)
                                 + proj_step(wk_sb, ktc[1], 1, 3, psF),
                       slots={1: 0, 3: 1, 5: 2, 7: 3})
                psF.release()
                psSB = tc.alloc_tile_pool(name="psSB", bufs=1, space="PSUM")
                psM = tc.alloc_tile_pool(name="psM", bufs=1, space="PSUM", side="right")
                wo_units = []
                for qt in range(8):
                    wo_units.append(lambda qt=qt: tail_t(qt, psM))
                    wo_units.append(lambda qt=qt: tail_o(qt, psM))
                # interleave t/o one-lagged: t0,t1,o0,t2,o1,...,t7,o6,o7
                seq = [wo_units[0], wo_units[2]]
                for i in range(6):
                    seq += [wo_units[2 * i + 1], wo_units[2 * i + 4]]
                seq += [wo_units[13], wo_units[15]]
                head_B(2, psSB, filler=seq[0:8], narrow=True,
                       slots={1: 0, 3: 1, 5: 2, 7: 3, 9: 4, 11: 5,
                              13: 6, 14: 7}, psM=psM)
                head_B(3, psSB, filler=seq[8:16], narrow=True,
                       slots={i: i for i in range(8)},
                       fuse_out=True, psM=psM)
                psSB.release()
                psM.release()
                psA.release()
                sbH.release()

    nc.compile()
    return nc


def _host_inputs(x, token_positions, Wq, Wk, Wv, Wo):
    import ml_dtypes
    bf = ml_dtypes.bfloat16

    def pack(a, nblk):
        # [nblk*128, w] -> [128, nblk*w] with block d at cols [d*w, (d+1)*w)
        w = a.shape[1]
        return np.ascontiguousarray(
            a.reshape(nblk, 128, w).transpose(1, 0, 2).reshape(128, nblk * w))

    def pack_emajor(a):
        # a: [1024 (d rows), 256 (e cols)] -> [128, e*1024 + d*128 + j]
        return np.ascontiguousarray(
            a.reshape(8, 128, 2, 128).transpose(1, 2, 0, 3).reshape(128, 2048))

    perm = np.concatenate([np.arange(0, DK, 2), np.arange(1, DK, 2)])
    inv_freq = THETA ** (-np.arange(0, DK, 2, dtype=np.float64) / DK)
    swap32 = np.zeros((128, 128), np.float32)
    swap32[np.arange(128) ^ 32, np.arange(128)] = 1.0
    trid = np.concatenate(
        [np.triu(np.ones((128, 128), np.float32)),
         np.eye(128, dtype=np.float32), swap32],
        axis=1)
    in_maps = []
    for c in range(NCORES):
        b, g = divmod(c, NCORES // B)
        heads = [(g * HPC + h) for h in range(HPC)]
        rows_rope = np.concatenate([h * DK + perm for h in heads])
        rows_plain = np.concatenate([h * DK + np.arange(DK) for h in heads])
        pos = token_positions[b].astype(np.float64)
        ang = inv_freq[:, None] * pos[None, :]  # [32, S]
        cosv = np.cos(ang).astype(np.float32)
        sinv = np.sin(ang).astype(np.float32)
        cosF = np.concatenate([cosv] * 4, axis=0)  # [128, S]
        sinF = np.concatenate([sinv, -sinv, sinv, -sinv], axis=0)
        in_maps.append({
            "x_d": pack(np.ascontiguousarray(x[b].T), DT8).astype(bf),
            "wq_d": pack_emajor(np.ascontiguousarray(Wq[rows_rope, :].T)).astype(bf),
            "wk_d": pack_emajor(np.ascontiguousarray(Wk[rows_rope, :].T)).astype(bf),
            "wv_d": pack(np.ascontiguousarray(Wv[rows_plain, :].T), DT8).astype(bf),
            "wo_d": pack(np.ascontiguousarray(Wo[:, rows_plain].T), 2).astype(bf),
            "cos_d": cosF.astype(bf),
            "sin_d": sinF.astype(bf),
            "trid_d": trid.astype(bf),
        })
    return in_maps


def kernel(x, token_positions, Wq, Wk, Wv, Wo, _debug=False):
    x = np.asarray(x, np.float32)
    token_positions = np.asarray(token_positions, np.int32)
    Wq, Wk, Wv, Wo = (np.asarray(w, np.float32) for w in (Wq, Wk, Wv, Wo))
    if "nc" not in _cache:
        _cache["nc"] = _build_nc()
    nc = _cache["nc"]
    in_maps = _host_inputs(x, token_positions, Wq, Wk, Wv, Wo)
    res = run_bass_kernel_spmd(
        nc, in_maps, core_ids=list(range(NCORES)), trace=False)
    outs = [np.asarray(r["out"], np.float32) for r in res.results]
    full = np.zeros((B, S, D), np.float32)
    for c in range(NCORES):
        full[c // (NCORES // B)] += outs[c]
    if _debug:
        return full, res
    return full
